# revision 5
# baseline (speedup 1.0000x reference)
"""GATv2 Bass kernel v2 for Trainium2, 8 NeuronCores.

Problem: B=2, N=512, FIN=128, H=4, D=64 GATv2 attention (dense graph).
Sharding: one (batch, head) pair per core (B*H = 8 = n_cores).

Math per (b, h):
  h[n] = x[n] @ Wp + bp
  zi[n,e] = h[n]·W1[e]; zj[n,e] = h[n]·W2[e]; v_ije = zi[i,e]+zj[j,e]+bc[e]
  score[i,j] = sum_e a_e lrelu(v); attn = softmax_j; out = attn @ h

ReLU decomposition (v2): lrelu(v) = v + 0.8 ReLU(-v), so
  score[i,j] = A_i + B_j + sum_e s_e ReLU(u_i(e) + y_j(e))
  u_i(e) = -0.8|a_e| zi[i,e] (+bias), y_j(e) = -0.8|a_e| (zj[j,e]+bc_e)
  s_e = sign(a_e); A_i constant per row -> drops in softmax.
Kernel: e lives on PARTITIONS. Ydup [128,512] = yT stacked twice (bf16).
Per row-pair (2 rows per 128 partitions): W = ReLU(Ydup + u-col) via ONE
fused DVE tensor_scalar (add,max) at 2-4x rate, or ACT activation(Relu,
bias). PE contracts with +-1 stationaries [128,32] (16 pairs accumulate
into one 32-partition stripe of the score bank; tile_position picks the
stripe). B_j added exactly via a [2,128]x[2,512] accumulating matmul with
B split hi/lo in bf16. Softmax reads scores straight from PSUM; epilogue
(attn @ h with h split hi/lo bf16) as in v1. Softmax/epilogue emission is
deferred by one i-block so the strict-FIFO DVE/ACT queues never stall.
"""

import numpy as np
import ml_dtypes

import concourse.bacc as bacc
import concourse.mybir as mybir
import concourse.tile as tile
from concourse.bass_utils import run_bass_kernel_spmd

F32 = mybir.dt.float32
BF16 = mybir.dt.bfloat16
BF = ml_dtypes.bfloat16

B, N, FIN, H, D = 2, 512, 128, 4, 64
NEG_SLOPE = 0.2
E = D
NB = N // 128

last_results = None

_cache = {}


def _build(use_bias_param):
    nc = bacc.Bacc("TRN2", target_bir_lowering=False, debug=False,
                   num_devices=8)

    x_d = nc.dram_tensor("x", [N, FIN], F32, kind="ExternalInput")
    id128_d = nc.dram_tensor("id128", [128, 128], F32, kind="ExternalInput")
    mm_d = nc.dram_tensor("mm", [128, 2 * E], BF16, kind="ExternalInput")
    pks_d = nc.dram_tensor("pks", [128, 3], F32, kind="ExternalInput")
    wp_d = nc.dram_tensor("wproj", [FIN, D], F32, kind="ExternalInput")
    s16_d = nc.dram_tensor("s16", [128, 512], BF16, kind="ExternalInput")
    id128b_d = nc.dram_tensor("id128b", [128, 128], BF16,
                              kind="ExternalInput")
    if use_bias_param:
        bprm_d = nc.dram_tensor("biasprm", [128, D], F32,
                                kind="ExternalInput")
    out_d = nc.dram_tensor("out", [N, D], F32, kind="ExternalOutput")

    AF = mybir.ActivationFunctionType
    ALU = mybir.AluOpType
    AX = mybir.AxisListType

    with tile.TileContext(nc) as tc:
        with tc.tile_pool(name="sb", bufs=1) as sb:
            xb = sb.tile([128, NB * 128], F32)
            xT = sb.tile([128, N], F32)
            xTb = sb.tile([128, N], BF16)
            id128 = sb.tile([128, 128], F32)
            mmt = sb.tile([128, 2 * E], BF16)
            pks = sb.tile([128, 3], F32)
            sBb = sb.tile([E, 1], BF16)
            s16 = sb.tile([128, 512], BF16)
            wp = sb.tile([FIN, D], F32)
            id128b = sb.tile([128, 128], BF16)
            ydup = sb.tile([128, N], BF16)
            zidup = sb.tile([128, NB * 64], F32)
            yTsb = sb.tile([E, N], BF16)
            h_hi = sb.tile([128, NB * (D + 1)], BF16)
            h_lo = sb.tile([128, NB * (D + 1)], BF16)
            h_f = sb.tile([128, NB * D], F32)
            bpf = sb.tile([1, N], F32)
            expBc = sb.tile([128, NB], F32)
            ee = [sb.tile([128, N], BF16, tag=f"ee{i}", name=f"ee{i}")
                  for i in range(NB)]
            rowmax = [sb.tile([128, 1], F32, tag=f"rm{i}", name=f"rm{i}")
                      for i in range(NB)]
            negm = [sb.tile([128, 1], F32, tag=f"nm{i}", name=f"nm{i}")
                    for i in range(NB)]
            zsum = [sb.tile([128, 1], F32, tag=f"zs{i}", name=f"zs{i}")
                    for i in range(NB)]
            rz = [sb.tile([128, 1], F32, tag=f"rz{i}", name=f"rz{i}")
                  for i in range(NB)]
            if use_bias_param:
                bprm = sb.tile([128, D], F32)

            # ---------- input DMAs: spread across engine DGEs ----------
            for nb in range(NB):
                nc.sync.dma_start(
                    xb[:, nb * 128:(nb + 1) * 128],
                    x_d.ap()[nb * 128:(nb + 1) * 128, :])
            nc.scalar.dma_start(id128[:, 0:64], id128_d.ap()[:, 0:64])
            nc.scalar.dma_start(id128[:, 64:128], id128_d.ap()[:, 64:128])
            nc.gpsimd.dma_start(s16[:, 0:256], s16_d.ap()[:, 0:256])
            nc.gpsimd.dma_start(s16[:, 256:512], s16_d.ap()[:, 256:512])
            nc.scalar.dma_start(mmt[:], mm_d.ap())
            nc.scalar.dma_start(pks[:], pks_d.ap())
            nc.gpsimd.dma_start(wp[:], wp_d.ap())
            nc.gpsimd.dma_start(id128b[:], id128b_d.ap())
            if use_bias_param:
                nc.scalar.dma_start(bprm[:], bprm_d.ap())


            # ---------- prep ----------
            with tc.tile_pool(name="pp", bufs=4, space="PSUM") as pp:
                for nb in range(NB):
                    t = pp.tile([128, 512], F32, tag="t")
                    nc.tensor.transpose(t[:, 0:128],
                                        xb[:, nb * 128:(nb + 1) * 128],
                                        id128[:])
                    nc.scalar.copy(xT[:, nb * 128:(nb + 1) * 128],
                                   t[:, 0:128])
                    nc.vector.tensor_copy(xTb[:, nb * 128:(nb + 1) * 128],
                                          t[:, 0:128])
                nc.scalar.copy(sBb[:], pks[0:E, 2:3])
                # yT = M2 @ xT + c2 (bf16 mm) FIRST: it gates ydup -> maps
                yt = pp.tile([128, 512], F32, tag="t")
                nc.tensor.matmul(yt[0:E, :], mmt[:, E:2 * E], xTb[:],
                                 start=True, stop=True)
                # uT = M1 @ xT + c1 (bf16 mm), scattered into zidup (f32)
                ut = pp.tile([128, 512], F32, tag="t")
                nc.tensor.matmul(ut[0:E, :], mmt[:, 0:E], xTb[:],
                                 start=True, stop=True)
                # DVE: zidup scatter (fused add of c1) straight from PSUM
                utv = ut[0:E, :].rearrange("p (b q t) -> p b q t",
                                           b=NB, q=64, t=2)
                zde = zidup[0:E, :].rearrange("p (b q t) -> p b q t",
                                              b=NB, q=64, t=1)
                zdo = zidup[E:128, :].rearrange("p (b q t) -> p b q t",
                                                b=NB, q=64, t=1)
                nc.vector.tensor_scalar(zde[:, :, :, :], utv[:, :, :, 0:1],
                                        pks[0:E, 0:1], None, op0=ALU.add)
                nc.vector.tensor_scalar(zdo[:, :, :, :], utv[:, :, :, 1:2],
                                        pks[0:E, 0:1], None, op0=ALU.add)
                # ACT: ydup (gates the relu maps), then yTsb / b2
                nc.scalar.activation(ydup[0:E, :], yt[0:E, :], AF.Identity,
                                     bias=pks[0:E, 1:2])
                nc.scalar.activation(ydup[E:128, :], yt[0:E, :], AF.Identity,
                                     bias=pks[0:E, 1:2])
                nc.scalar.activation(yTsb[:], yt[0:E, :], AF.Identity,
                                     bias=pks[0:E, 1:2])

            # ---------- main: per i-block scores, softmax/epilogue of the
            # previous block interleaved at the HEAD of each block so the
            # strict-FIFO ACT/DVE queues never sit behind a full block of
            # relu maps ----------
            banks = []
            accs = {}
            eTs = {}
            hts = []
            bps = []

            def emit_map(ib, q, on_act):
                w = wpool.tile([128, N], BF16, tag="w")
                col = ib * 64 + q
                if on_act:
                    with nc.allow_low_precision(reason="bf16 relu"):
                        nc.scalar.activation(w[:], ydup[:], AF.Relu,
                                             bias=zidup[:, col:col + 1])
                else:
                    with nc.allow_low_precision(reason="bf16 relu"):
                        nc.vector.tensor_scalar(
                            w[:], ydup[:], zidup[:, col:col + 1],
                            0.0, op0=ALU.add, op1=ALU.max)
                st, m = divmod(q, 16)
                nc.tensor.matmul(
                    bank[32 * st:32 * st + 32, :],
                    s16[:, 32 * m:32 * m + 32], w[:],
                    start=(m == 0), stop=(m == 15),
                    tile_position=(0, 32 * st),
                    skip_group_check=True)

            def act_map(q):
                # ACT handles every 4th map from q=15 on; the head of each
                # block is DVE-only so ACT can run the previous block's
                # exp/epilogue without stalling the PE map stream
                return q >= 12 and q % 4 == 3

            def emit_block(ib, prev):
                global bank
                bank = scp.tile([128, N], F32, tag="bank", name=f"bank{ib}")
                banks.append(bank)
                for q in range(64):
                    emit_map(ib, q, act_map(q))
                    if prev is None:
                        if q == 2:
                            hall = epp.tile([128, NB * D], F32, tag="hall",
                                            bufs=1)
                            hts.append(hall)
                            for nb in range(NB):
                                nc.tensor.matmul(
                                    hall[:, nb * D:(nb + 1) * D],
                                    xT[:, nb * 128:(nb + 1) * 128],
                                    wp[:], start=True, stop=True,
                                    skip_group_check=True)
                        elif q == 3:
                            bp_ = epp.tile([128, 512], F32, tag="bp",
                                           bufs=1)
                            bps.append(bp_)
                            nc.tensor.matmul(bp_[0:1, :], sBb[:], yTsb[:],
                                             start=True, stop=True)
                        elif q == 4:
                            nc.scalar.copy(bpf[:], bps[0][0:1, :])
                        elif q == 5:
                            # transpose B row chunks into columns [128, NB]
                            bt = epp.tile([128, NB], F32, tag="bt", bufs=1)
                            bps.append(bt)
                            for c in range(NB):
                                nc.tensor.transpose(
                                    bt[:, c:c + 1],
                                    bpf[0:1, c * 128:(c + 1) * 128],
                                    id128[0:1, 0:1])
                        elif q == 7:
                            nc.scalar.activation(expBc[:], bps[1][:, :],
                                                 AF.Exp, bias=0.0)
                        elif q == 9:
                            for nb in range(NB):
                                nc.scalar.activation(
                                    h_hi[:, nb * 65:nb * 65 + D],
                                    hts[0][:, nb * D:(nb + 1) * D],
                                    AF.Copy, bias=0.0,
                                    scale=expBc[:, nb:nb + 1])
                                nc.scalar.activation(
                                    h_f[:, nb * D:(nb + 1) * D],
                                    hts[0][:, nb * D:(nb + 1) * D],
                                    AF.Copy, bias=0.0,
                                    scale=expBc[:, nb:nb + 1])
                        elif q == 11:
                            for nb in range(NB):
                                nc.scalar.copy(h_hi[:, nb * 65 + D:
                                                    nb * 65 + D + 1],
                                               expBc[:, nb:nb + 1])
                        elif q == 24:
                            for nb in range(NB):
                                nc.vector.tensor_tensor(
                                    h_lo[:, nb * 65:nb * 65 + D],
                                    h_f[:, nb * D:(nb + 1) * D],
                                    h_hi[:, nb * 65:nb * 65 + D],
                                    op=ALU.subtract)
                            for nb in range(NB):
                                nc.vector.tensor_tensor(
                                    h_lo[:, nb * 65 + D:nb * 65 + D + 1],
                                    expBc[:, nb:nb + 1],
                                    h_hi[:, nb * 65 + D:nb * 65 + D + 1],
                                    op=ALU.subtract)
                        continue
                    if q == 7:
                        # ACT exp for prev block (uncentered: scores bounded)
                        pb = banks[prev]
                        nc.scalar.activation(ee[prev][:], pb[:, :], AF.Exp,
                                             bias=0.0)
                    elif q == 9:
                        eT = sb.tile([128, 128 * NB], BF16, tag=f"eT{prev}",
                                     name=f"eT{prev}")
                        eTs[prev] = eT
                        for jb in range(NB):
                            t = epp.tile([128, 128], BF16, tag="et")
                            nc.tensor.transpose(
                                t[:], ee[prev][:, jb * 128:(jb + 1) * 128],
                                id128b[:])
                            nc.scalar.copy(
                                eT[:, jb * 128:(jb + 1) * 128], t[:])
                    elif q == 13:
                        eT = eTs[prev]
                        acc = epp.tile([128, D + 1], F32, tag="acc",
                                       name=f"acc{prev}", bufs=1)
                        accs[prev] = acc
                        for jb in range(NB):
                            nc.tensor.matmul(
                                acc[:], eT[:, jb * 128:(jb + 1) * 128],
                                h_hi[:, jb * 65:(jb + 1) * 65],
                                start=(jb == 0), stop=False)
                            nc.tensor.matmul(
                                acc[:], eT[:, jb * 128:(jb + 1) * 128],
                                h_lo[:, jb * 65:(jb + 1) * 65],
                                start=False, stop=(jb == NB - 1))
                    elif q == 15:
                        nc.vector.reciprocal(rz[prev][:],
                                             accs[prev][:, D:D + 1])
                    elif q == 17:
                        emit_out(prev)

            def emit_out(ib):
                o = sb.tile([128, D], F32, tag=f"o{ib}", name=f"o{ib}")
                nc.scalar.activation(o[:], accs[ib][:, 0:D], AF.Copy,
                                     bias=0.0, scale=rz[ib][:, 0:1])
                if use_bias_param:
                    nc.vector.tensor_tensor(o[:], o[:], bprm[:], op=ALU.add)
                nc.gpsimd.dma_start(out_d.ap()[ib * 128:(ib + 1) * 128, :],
                                    o[:])

            with tc.tile_pool(name="scores", bufs=2, space="PSUM") as scp, \
                 tc.tile_pool(name="wpl", bufs=12) as wpool, \
                 tc.tile_pool(name="ep", bufs=2, space="PSUM") as epp:
                for ib in range(NB):
                    emit_block(ib, ib - 1 if ib >= 1 else None)
                # tail: last block's softmax + epilogue
                lb = NB - 1
                pb = banks[lb]
                nc.scalar.activation(ee[lb][:], pb[:, :], AF.Exp,
                                     bias=0.0)
                eT = sb.tile([128, 128 * NB], BF16, tag=f"eT{lb}",
                             name=f"eT{lb}")
                for jb in range(NB):
                    t = epp.tile([128, 128], BF16, tag="et")
                    nc.tensor.transpose(
                        t[:], ee[lb][:, jb * 128:(jb + 1) * 128], id128b[:])
                    if jb % 2 == 0:
                        nc.vector.tensor_copy(
                            eT[:, jb * 128:(jb + 1) * 128], t[:])
                    else:
                        nc.scalar.copy(eT[:, jb * 128:(jb + 1) * 128], t[:])
                acc = epp.tile([128, D + 1], F32, tag="acc", name=f"acc{lb}",
                               bufs=1)
                accs[lb] = acc
                for jb in range(NB):
                    nc.tensor.matmul(
                        acc[:], eT[:, jb * 128:(jb + 1) * 128],
                        h_hi[:, jb * 65:(jb + 1) * 65],
                        start=(jb == 0), stop=False)
                    nc.tensor.matmul(
                        acc[:], eT[:, jb * 128:(jb + 1) * 128],
                        h_lo[:, jb * 65:(jb + 1) * 65],
                        start=False, stop=(jb == NB - 1))
                nc.vector.reciprocal(rz[lb][:], acc[:, D:D + 1])
                emit_out(lb)

    nc.compile()
    return nc


def kernel(x, W_proj, b_proj, W_cat_weight, W_cat_bias, a, bias_param):
    global last_results
    x = np.asarray(x, dtype=np.float32)
    W_proj = np.asarray(W_proj, dtype=np.float32)
    b_proj = np.asarray(b_proj, dtype=np.float32)
    W_cat_weight = np.asarray(W_cat_weight, dtype=np.float32)
    W_cat_bias = np.asarray(W_cat_bias, dtype=np.float32)
    a = np.asarray(a, dtype=np.float32)
    bias_param = np.asarray(bias_param, dtype=np.float32)

    W1 = W_cat_weight[:, :, :D]
    W2 = W_cat_weight[:, :, D:]

    use_bias_param = bool(np.any(bias_param))
    key = (use_bias_param,)
    if key not in _cache:
        _cache[key] = _build(*key)
    nc = _cache[key]

    id128 = np.eye(128, dtype=np.float32)
    s16 = None  # per-core below

    in_maps = []
    for c in range(8):
        b, hh = divmod(c, H)
        ah = a[hh]
        s = np.sign(ah).astype(np.float32)
        abs_a = np.abs(ah)
        Wp = W_proj[hh]
        bp = b_proj[hh]
        bc = W_cat_bias[hh]
        M1 = -0.8 * (abs_a[:, None] * W1[hh]) @ Wp.T     # [E, FIN]
        c1 = -0.8 * abs_a * (W1[hh] @ bp)                # [E]
        M2 = -0.8 * (abs_a[:, None] * W2[hh]) @ Wp.T     # [E, FIN]
        c2 = -0.8 * abs_a * (W2[hh] @ bp + bc)           # [E]
        mmt = np.concatenate([M1.T, M2.T], axis=1)       # [FIN, 2E]
        pks = np.zeros((128, 3), dtype=np.float32)
        pks[0:E, 0] = c1
        pks[0:E, 1] = c2
        pks[0:E, 2] = -1.25 * s
        s16 = np.zeros((128, 512), dtype=np.float32)
        for m in range(16):
            s16[0:E, 32 * m + 2 * m] = s
            s16[E:128, 32 * m + 2 * m + 1] = s
        m = {
            "x": np.ascontiguousarray(x[b]),
            "id128": id128,
            "mm": np.ascontiguousarray(mmt).astype(BF),
            "pks": pks,
            "wproj": np.ascontiguousarray(Wp),
            "s16": s16.astype(BF),
            "id128b": id128.astype(BF),
        }
        if use_bias_param:
            m["biasprm"] = np.tile(bias_param[None, hh * D:(hh + 1) * D],
                                   (128, 1)).astype(np.float32)
        in_maps.append(m)

    res = run_bass_kernel_spmd(nc, in_maps, core_ids=list(range(8)))
    last_results = res

    out = np.empty((B, N, H * D), dtype=np.float32)
    for c in range(8):
        b, hh = divmod(c, H)
        out[b, :, hh * D:(hh + 1) * D] = res.results[c]["out"]
    return out
